# revision 16
# baseline (speedup 1.0000x reference)
"""DPQ (gumbel-softmax product-quantizer autoencoder) forward pass on 8
Trainium2 NeuronCores, data-parallel over the batch dimension.

Math (per row n, subspace m of 8, codebook of K=512 64-dim codes):
  h = x @ W_enc + b_enc                     [N, M*DSUB]
  score = (-|h|^2 + 2 h.c - |c|^2) / T_m    squared-distance scores
  codes = softmax(score + gumbel)           gumbel-softmax, TAU=1
  y = (codes @ C).flatten() @ W_dec + b_dec

Implementation notes:
  * The -|h|^2 term is constant over k, so softmax cancels it exactly; it is
    never computed. No max-subtraction either: z = (2 h.c - |c|^2)/T + g is
    bounded (~+45 max on these inputs), exp() stays in fp32 range.
  * Everything runs transposed (n on the free dim): the host pre-transposes
    x, gumbel and the codebook so no PE transposes are needed anywhere.
    The output is produced transposed and the host untransposes it.
  * Gumbel noise is pre-cast to bf16 on the host (halves its DMA traffic)
    and added into the score PSUM accumulation group by a single identity
    matmul per [128,512] tile.
  * All matmuls run in fp32r (11-bit mantissa, 1 cycle/row vs fp32's 4) as
    single products; end-to-end rel_absmax vs the fp32 reference is ~3e-3
    (validated in numpy simulation), well inside the 2e-2 gate.
  * The softmax denominator s rides along as a ones-column appended to the
    codebook in the recon matmul (row 64 of the U PSUM tile). The DVE
    RECIPROCAL runs at ~8 cycles/element (iterative divide) and its cost
    scales with the free dim only, so the 16 per-block 1/s rows are staged
    at 32-aligned partitions of two ones-initialized [97,512] tiles and
    inverted in two batched reciprocals per block; 1/s is then broadcast
    across partitions by one selector-matmul per subspace pair.
  * The encoder and first score-pair of block b+1 are issued before the 1/s
    tail of block b so the PE never idles long enough for the HAM clock
    gate to re-throttle (the throttle cost the first version ~350us).
"""

import sys
sys.path.insert(0, '/opt/trn_rl_repo')

import numpy as np
import ml_dtypes

N, D, M, K, DSUB = 32768, 512, 8, 512, 64
NCORES = 8
NLOC = N // NCORES          # rows per core
BLK = 512                   # rows per block
JC = D // 128               # 4 column chunks of 128
KC = K // 128               # 4 code chunks of 128
MC = M // 2                 # 4 subspace pairs

_CACHE = {}


def _f32r_round(x: np.ndarray) -> np.ndarray:
    """Round fp32 -> fp32r (11-bit mantissa; idempotent under HW rounding)."""
    b = np.ascontiguousarray(x, dtype=np.float32).view(np.uint32).copy()
    b += 0x800
    b &= 0xFFFFF000
    return b.view(np.float32)


def build(nblk: int):
    import concourse.bacc as bacc_mod
    import concourse.tile as tile
    import concourse.mybir as mybir
    from concourse.bass import ts
    from concourse.masks import make_identity
    from contextlib import ExitStack

    F32 = mybir.dt.float32
    F32R = mybir.dt.float32r
    BF16 = mybir.dt.bfloat16
    AF = mybir.ActivationFunctionType
    ALU = mybir.AluOpType

    nloc = nblk * BLK
    nc = bacc_mod.Bacc(trn_type="TRN2", target_bir_lowering=False, debug=False)

    # Host-prepared layouts (see make_in_maps):
    #   XT[p, b, dc, j]      = f32r(x[b*512+j, dc*128+p])
    #   GT[m, p, b, kc, j]   = bf16(g[b*512+j, m, kc*128+p])
    #   YT[p, b, jc, j]      = y[b*512+j, jc*128+p]            (output)
    #   CBT[m, d, k]         = codebook[m, k, d]
    XT = nc.dram_tensor("xt", [128, nblk, JC, BLK], F32R, kind="ExternalInput").ap()
    WENC = nc.dram_tensor("w_enc", [D, D], F32R, kind="ExternalInput").ap()
    BENC = nc.dram_tensor("b_enc", [D], F32, kind="ExternalInput").ap()
    # CBP[p, m, kc, d] = codebook[m, kc*128+p, d]  (8 KB/partition lines)
    CBP = nc.dram_tensor("codebook", [128, M, KC, DSUB], F32,
                         kind="ExternalInput").ap()
    CBT = nc.dram_tensor("codebook_t", [M, DSUB, K], F32, kind="ExternalInput").ap()
    LOGT = nc.dram_tensor("log_t", [1, M], F32, kind="ExternalInput").ap()
    GT = nc.dram_tensor("gumbel", [M, 128, nblk, KC, BLK], BF16,
                        kind="ExternalInput").ap()
    WDEC = nc.dram_tensor("w_dec", [D, D], F32R, kind="ExternalInput").ap()
    BDEC = nc.dram_tensor("b_dec", [D], F32, kind="ExternalInput").ap()
    YT = nc.dram_tensor("yt", [128, nblk, JC, BLK], BF16, kind="ExternalOutput").ap()

    with tile.TileContext(nc) as tc, ExitStack() as ctx:
        cst = ctx.enter_context(tc.tile_pool(name="cst", bufs=1))
        sb = ctx.enter_context(tc.tile_pool(name="sb", bufs=2))
        ps = ctx.enter_context(tc.tile_pool(name="ps", bufs=8, space="PSUM"))

        def bank(tag, bufs, parts=128, free=BLK):
            return ps.tile([parts, free], F32, tag=tag, bufs=bufs, name="bank")

        # ---------------- prologue: constants & weights ----------------
        ident = cst.tile([128, 128], F32, tag="ident")
        make_identity(nc, ident[:])
        identb = cst.tile([128, 128], BF16, tag="identb")
        nc.vector.tensor_copy(identb[:], ident[:])
        ones_f = cst.tile([128, 128], F32, tag="ones_f")
        nc.gpsimd.memset(ones_f[:], 1.0)
        ones128_r = cst.tile([1, 128], F32R, tag="o128r")
        nc.vector.tensor_copy(ones128_r[:], ones_f[0:1, :])
        # selector weights for the paired 1/s broadcast: sel.T @ rec puts the
        # rec row at partition 32a on out partitions 0..63 and the row at
        # partition 32a+32 on 64..127 (a=0 for sel_lo, a=2 for sel_hi).
        sel_f = cst.tile([97, 2, 128], F32, tag="sel_f")
        nc.gpsimd.memset(sel_f[:], 0.0)
        nc.gpsimd.memset(sel_f[0:1, 0, 0:64], 1.0)
        nc.gpsimd.memset(sel_f[32:33, 0, 64:128], 1.0)
        nc.gpsimd.memset(sel_f[64:65, 1, 0:64], 1.0)
        nc.gpsimd.memset(sel_f[96:97, 1, 64:128], 1.0)
        sel_lo = cst.tile([97, 128], F32R, tag="sel_lo")
        nc.vector.tensor_copy(sel_lo[:], sel_f[:, 0, :])
        sel_hi = cst.tile([97, 128], F32R, tag="sel_hi")
        nc.vector.tensor_copy(sel_hi[:], sel_f[:, 1, :])

        # temperatures: invT = exp(-logT) [1, 8] -> bcast to [128, 8]
        # (first ACT op in the program: loads the exp table set once)
        logt = cst.tile([1, M], F32, tag="logt")
        nc.sync.dma_start(logt[:], LOGT)
        invt = cst.tile([1, M], F32, tag="invt")
        nc.scalar.activation(invt[:], logt[:], AF.Exp, bias=0.0, scale=-1.0)
        invt_r = cst.tile([1, M], F32R, tag="invt_r")
        nc.vector.tensor_copy(invt_r[:], invt[:])
        ibp = bank("mis", 2, 128, M)
        nc.tensor.matmul(ibp[:], lhsT=ones128_r[:], rhs=invt_r[:],
                         start=True, stop=True)
        sc2 = cst.tile([128, M], F32, tag="sc2")    # 2*invT per partition
        nc.vector.tensor_scalar_mul(sc2[:], ibp[:], 2.0)
        scn = cst.tile([128, M], F32, tag="scn")    # -invT per partition
        nc.vector.tensor_scalar_mul(scn[:], ibp[:], -1.0)

        # encoder / decoder biases as [128,1] per column chunk
        benc2 = BENC.rearrange("(a b) -> a b", b=1)
        bdec2 = BDEC.rearrange("(a b) -> a b", b=1)
        benc_c, bdec_c = [], []
        for jc in range(JC):
            bet = cst.tile([128, 1], F32, tag=f"benc{jc}", name="bet")
            nc.sync.dma_start(bet[:], benc2[ts(jc, 128), :])
            benc_c.append(bet)
            bdt = cst.tile([128, 1], F32, tag=f"bdec{jc}", name="bdt")
            nc.sync.dma_start(bdt[:], bdec2[ts(jc, 128), :])
            bdec_c.append(bdt)

        # encoder weights early: block 0's encoder only needs these + x
        wenc = cst.tile([128, JC, D], F32R, tag="wenc")
        nc.sync.dma_start(wenc[:], WENC.rearrange("(c p) j -> p c j", p=128))

        # ---------------- main loop over row blocks ----------------
        # Per-block state: hr (4 encoder tiles), sAB (2 s-staging tiles),
        # upss (4 staged recon tiles).
        state = {}

        def encoder(b):
            """h^T = W_enc^T x^T + b for block b; also inits s staging."""
            xt = sb.tile([128, JC, BLK], F32R, tag="xt", bufs=2, name="xt")
            nc.sync.dma_start(xt[:], XT[:, b])
            hr = []
            for jc in range(JC):
                hp = bank("mis", 2)
                for dc in range(JC):
                    nc.tensor.matmul(hp[:], lhsT=wenc[:, dc, ts(jc, 128)],
                                     rhs=xt[:, dc, :], start=(dc == 0),
                                     stop=(dc == JC - 1))
                hrt = sb.tile([128, BLK], F32R, tag=f"hr{jc}", bufs=2,
                              name="hrt")
                nc.scalar.activation(hrt[:], hp[:], AF.Identity,
                                     bias=benc_c[jc][:, 0:1], scale=1.0)
                hr.append(hrt)
            sAB = []
            for t2 in range(2):
                sT = sb.tile([97, BLK], F32, tag=f"sAB{t2}", bufs=2, name="sT")
                nc.gpsimd.memset(sT[:], 1.0)
                sAB.append(sT)
            state[b] = dict(hr=hr, sAB=sAB, upss=[None] * MC)

        def load_gt(b, mc):
            gt0 = sb.tile([128, KC, BLK], BF16, tag="gt0", bufs=2, name="gt")
            nc.sync.dma_start(gt0[:], GT[2 * mc, :, b])
            gt1 = sb.tile([128, KC, BLK], BF16, tag="gt1", bufs=2, name="gt")
            nc.sync.dma_start(gt1[:], GT[2 * mc + 1, :, b])
            return gt0, gt1

        def score_pair(b, mc, gts=None):
            """scores -> codes -> unnormalized recon for pair mc of block b;
            stages the recon and the softmax denominators out to SBUF."""
            st = state[b]
            m0, m1 = 2 * mc, 2 * mc + 1
            gt0, gt1 = gts if gts is not None else load_gt(b, mc)
            up0 = bank("up", 2, 65)
            up1 = bank("up", 2, 65)
            for kc in range(KC):
                zp0 = bank("zp", 4)
                zp1 = bank("zp", 4)
                # paired 64-row score matmuls (adjacent issue -> the PE
                # runs the two row-strips concurrently)
                nc.tensor.matmul(zp0[:], lhsT=ct2[(mc, kc)][0:64, :],
                                 rhs=st["hr"][mc][0:64, :], start=True,
                                 stop=False, tile_position=(0, 0))
                nc.tensor.matmul(zp1[:], lhsT=ct2[(mc, kc)][64:128, :],
                                 rhs=st["hr"][mc][64:128, :], start=True,
                                 stop=False, tile_position=(64, 0))
                # gumbel noise: one identity-matmul each into the same
                # accumulation group
                nc.tensor.matmul(zp0[:], lhsT=identb[:],
                                 rhs=gt0[:, kc, :], start=False, stop=True)
                nc.tensor.matmul(zp1[:], lhsT=identb[:],
                                 rhs=gt1[:, kc, :], start=False, stop=True)
                cf0 = sb.tile([128, BLK], F32R, tag=f"cf0_{kc}", bufs=2,
                              name="cf")
                nc.scalar.activation(cf0[:], zp0[:], AF.Exp,
                                     bias=bias_mk[(m0, kc)][:, 0:1],
                                     scale=1.0)
                cf1 = sb.tile([128, BLK], F32R, tag=f"cf1_{kc}", bufs=2,
                              name="cf")
                nc.scalar.activation(cf1[:], zp1[:], AF.Exp,
                                     bias=bias_mk[(m1, kc)][:, 0:1],
                                     scale=1.0)
                nc.tensor.matmul(up0[:], lhsT=cones[(m0, kc)][:],
                                 rhs=cf0[:], start=(kc == 0),
                                 stop=(kc == KC - 1))
                nc.tensor.matmul(up1[:], lhsT=cones[(m1, kc)][:],
                                 rhs=cf1[:], start=(kc == 0),
                                 stop=(kc == KC - 1))
            # stage recon + softmax denominators out of PSUM; s rows land
            # at 32-aligned partitions: m -> tile m//4, partition 32*(m%4)
            ups = sb.tile([128, BLK], F32, tag="ups", bufs=MC + 1, name="ups")
            nc.vector.tensor_copy(ups[0:64, :], up0[0:64, :])
            nc.vector.tensor_copy(ups[64:128, :], up1[0:64, :])
            sT0 = st["sAB"][m0 // 4]
            r0 = 32 * (m0 % 4)
            nc.vector.tensor_copy(sT0[r0:r0 + 1, :], up0[64:65, :])
            sT1 = st["sAB"][m1 // 4]
            r1 = 32 * (m1 % 4)
            nc.vector.tensor_copy(sT1[r1:r1 + 1, :], up1[64:65, :])
            st["upss"][mc] = ups

        def tail(b):
            """batched 1/s, broadcast, recon scaling, decoder, store."""
            st = state.pop(b)
            recs = []
            for t2 in range(2):
                rc = sb.tile([97, BLK], F32R, tag=f"rec{t2}", bufs=2,
                             name="rc")
                with nc.allow_low_precision(reason="1/s in fp32r is plenty"):
                    nc.vector.reciprocal(rc[:], st["sAB"][t2][:])
                recs.append(rc)
            rth = []
            for mc in range(MC):
                bp = bank("mis", 2)
                nc.tensor.matmul(bp[:],
                                 lhsT=(sel_lo if mc % 2 == 0 else sel_hi)[:],
                                 rhs=recs[mc // 2][:], start=True, stop=True)
                rt = sb.tile([128, BLK], F32R, tag=f"rth{mc}", bufs=2,
                             name="rt")
                nc.vector.tensor_mul(rt[:], st["upss"][mc][:], bp[:])
                rth.append(rt)
            yo = sb.tile([128, JC, BLK], BF16, tag="yo", bufs=2, name="yo")
            for jc in range(JC):
                yp = bank("mis", 2)
                for mc in range(JC):
                    nc.tensor.matmul(yp[:], lhsT=wdec[:, mc, ts(jc, 128)],
                                     rhs=rth[mc][:], start=(mc == 0),
                                     stop=(mc == JC - 1))
                nc.scalar.activation(yo[:, jc, :], yp[:], AF.Identity,
                                     bias=bdec_c[jc][:, 0:1], scale=1.0)
            nc.sync.dma_start(YT[:, b], yo[:])

        # block 0's encoder + first gumbel pair go first so their DMAs are
        # not queued behind the 6 MB of decoder-weight/codebook prologue
        ct2 = {}
        cones = {}
        bias_mk = {}
        encoder(0)
        gts00 = load_gt(0, 0)

        # codebook prep:
        #   ct2[(mc, kc)] [128d2, 128k] fp32r = 2 invT_m C_m[kc]^T stacked for
        #     the (m even, m odd) pair on partition halves (score matmul lhsT)
        #   cones[(m, kc)] [128k, 65] fp32r = [C_m[kc] | ones]  (recon lhsT)
        #   bias[(m, kc)] [128k, 1] = -invT_m |c|^2              (ACT exp bias)
        cbts = cst.tile([64, M, K], F32, tag="cbts")
        nc.sync.dma_start(cbts[:], CBT.rearrange("m d k -> d m k"))
        cbs = cst.tile([128, M, KC, DSUB], F32, tag="cbs")
        nc.sync.dma_start(cbs[:], CBP)
        wdec = cst.tile([128, JC, D], F32R, tag="wdec")
        nc.sync.dma_start(wdec[:], WDEC.rearrange("(c p) j -> p c j", p=128))
        for m in range(M):
            half = (m % 2) * 64
            for kc in range(KC):
                if m % 2 == 0:
                    ct2[(m // 2, kc)] = cst.tile([128, 128], F32R,
                                                 tag=f"ct2_{m // 2}_{kc}",
                                                 name="ct2t")
                nc.vector.tensor_scalar_mul(
                    ct2[(m // 2, kc)][half:half + 64, :],
                    cbts[:, m, ts(kc, 128)], sc2[0:64, m:m + 1])
                chunk = cbs[:, m, kc, :]
                scrap = sb.tile([128, DSUB], F32, tag="cscrap", bufs=2,
                                name="scrap")
                n2 = sb.tile([128, 1], F32, tag="cn2", bufs=2, name="n2")
                nc.vector.scalar_tensor_tensor(scrap[:], chunk, 1.0, chunk,
                                               op0=ALU.mult, op1=ALU.mult,
                                               accum_out=n2[:])
                bt = cst.tile([128, 1], F32, tag=f"bias{m}_{kc}", name="bt")
                nc.vector.tensor_mul(bt[:], n2[:], scn[:, m:m + 1])
                bias_mk[(m, kc)] = bt
                ch_ = cst.tile([128, 65], F32R, tag=f"ch{m}_{kc}", name="ch_t")
                nc.vector.tensor_copy(ch_[:, 0:64], chunk)
                nc.vector.tensor_copy(ch_[:, 64:65], ones_f[:, 0:1])
                cones[(m, kc)] = ch_

        # software pipeline: the encoder + first score pair of block b+1 are
        # issued before tail(b) so the PE stays busy through the DVE
        # reciprocals and the HAM clock gate never re-throttles.
        score_pair(0, 0, gts00)
        for mc in range(1, MC):
            score_pair(0, mc)
        for b in range(nblk):
            if b + 1 < nblk:
                encoder(b + 1)
                score_pair(b + 1, 0)
            tail(b)
            if b + 1 < nblk:
                for mc in range(1, MC):
                    score_pair(b + 1, mc)

    nc.compile()
    return nc


def _get_nc(nblk: int):
    key = ("nc", nblk)
    if key not in _CACHE:
        _CACHE[key] = build(nblk)
    return _CACHE[key]


def make_in_maps(inputs: dict, nblk: int):
    nloc = nblk * BLK
    x = _f32r_round(inputs["x"])
    g = np.ascontiguousarray(inputs["gumbel_noise"], dtype=np.float32)
    cb = np.ascontiguousarray(inputs["codebook"], dtype=np.float32)
    shared = dict(
        w_enc=_f32r_round(inputs["W_enc"]),
        b_enc=np.ascontiguousarray(inputs["b_enc"], dtype=np.float32),
        # CBP[p, m, kc, d] = codebook[m, kc*128+p, d]
        codebook=np.ascontiguousarray(
            cb.reshape(M, KC, 128, DSUB).transpose(2, 0, 1, 3)),
        codebook_t=np.ascontiguousarray(cb.transpose(0, 2, 1)),
        log_t=np.ascontiguousarray(
            inputs["log_temperatures"], dtype=np.float32).reshape(1, M),
        w_dec=_f32r_round(inputs["W_dec"]),
        b_dec=np.ascontiguousarray(inputs["b_dec"], dtype=np.float32),
    )
    in_maps = []
    for c in range(NCORES):
        lo = c * NLOC
        xc = x[lo:lo + nloc]                       # [nloc, D]
        # XT[p, b, dc, j] = x[b*512+j, dc*128+p]
        xt = np.ascontiguousarray(
            xc.reshape(nblk, BLK, JC, 128).transpose(3, 0, 2, 1))
        gc = g[lo:lo + nloc]                       # [nloc, M, K]
        # GT[m, p, b, kc, j] = g[b*512+j, m, kc*128+p]
        gt = np.ascontiguousarray(
            gc.reshape(nblk, BLK, M, KC, 128).transpose(2, 4, 0, 3, 1)
        ).astype(ml_dtypes.bfloat16)
        in_maps.append(dict(shared, xt=xt, gumbel=gt))
    return in_maps


def run(inputs: dict, nblk: int = NLOC // BLK, trace: bool = False):
    from concourse.bass_utils import run_bass_kernel_spmd
    nc = _get_nc(nblk)
    in_maps = make_in_maps(inputs, nblk)
    res = run_bass_kernel_spmd(nc, in_maps, list(range(NCORES)), trace=trace)
    nloc = nblk * BLK
    out = np.empty((NCORES * nloc, D), dtype=np.float32)
    for c in range(NCORES):
        # YT[p, b, jc, j] -> y[b*512+j, jc*128+p]
        yt = np.asarray(res.results[c]["yt"], dtype=np.float32)
        out[c * nloc:(c + 1) * nloc] = np.ascontiguousarray(
            yt.transpose(1, 3, 2, 0).reshape(nloc, D))
    return out, res


def kernel(**inputs) -> np.ndarray:
    out, _ = run(inputs)
    return out
